# revision 17
# baseline (speedup 1.0000x reference)
"""DANetHead Trainium2 kernel.

8 NeuronCores = 2 samples x 4 row-blocks. Each core:
  - 3x3 convs (pam/cam heads, bf16 matmuls, fp32 accum) for its full sample
  - GroupNorm(32) + ReLU (exact fp32 stats)
  - PAM spatial attention for its 1152-position window (1024 center + 64 halo
    row each side), computed column-oriented: S[j,i] = E[j,i] (E symmetric,
    nonneg since features are post-ReLU), P = exp(S - 64) unnormalized,
    W = V @ P and Z = 1^T P via matmuls, normalize at the end.
  - CAM channel attention (full gram, row softmax, weighted values on window)
  - 1x1 predictor + bias
  - bilinear x4 upsample (phase-decomposed, DVE)
Host assembles the 8 output slices.
"""

import os
import sys

sys.path.insert(0, "/opt/trn_rl_repo")

import numpy as np
import ml_dtypes
from contextlib import ExitStack

import concourse.bass as bass
import concourse.tile as tile
from concourse import mybir
from concourse.bass import ds

F32 = mybir.dt.float32
BF16 = mybir.dt.bfloat16
AFT = mybir.ActivationFunctionType

B, CIN, H, W = 2, 256, 64, 64
CD, KC = 128, 19
N = H * W              # 4096
PADN = 64              # ext pad (one H row) each side
NEXT = N + 2 * PADN    # 4224
NH = 18 * 64           # 1152 window positions
SHIFT = 64.0           # softmax constant shift (valid: 0 <= E <= ~140)
EPS = 1e-5
NB = N // 128          # 32 j-blocks
SWEEPS = [(0, 512), (512, 512), (1024, 128)]  # (offset, width) over window

# bilinear x4 phase weights: out 4q+s uses (lo, hi) rows with these weights
C0 = [0.375, 0.125, 0.875, 0.625]   # weight on repA[:, q]
C1 = [0.625, 0.875, 0.125, 0.375]   # weight on repA[:, q+1]
# col phases: s'<2 reads padded cols (p', p'+1); s'>=2 reads (p'+1, p'+2)
CA = [0.375, 0.125, 0.875, 0.625]   # weight on first slice
CB = [0.625, 0.875, 0.125, 0.375]   # weight on second slice


def _build():
    nc = bass.Bass(trn_type="TRN2")
    xp_d = nc.dram_tensor("xp", [CIN, 66 * 66], BF16, kind="ExternalInput")
    wt_d = nc.dram_tensor("wt", [128, 2 * 2 * 9 * 128], BF16, kind="ExternalInput")
    cst_d = nc.dram_tensor("cst", [128, 10], F32, kind="ExternalInput")
    ind_d = nc.dram_tensor("ind", [128, 288], F32, kind="ExternalInput")
    cbf_d = nc.dram_tensor("cbf", [128, 148], BF16, kind="ExternalInput")
    out_d = nc.dram_tensor("outb", [4, 4, KC, 1024], F32, kind="ExternalOutput")

    with ExitStack() as ctx:
        tc = ctx.enter_context(tile.TileContext(nc))
        sing = ctx.enter_context(tc.tile_pool(name="sing", bufs=1))
        work = ctx.enter_context(tc.tile_pool(name="work", bufs=2))
        sm = ctx.enter_context(tc.tile_pool(name="sm", bufs=3))
        pt_pool = ctx.enter_context(tc.tile_pool(name="ptp", bufs=3))
        psA = ctx.enter_context(tc.tile_pool(name="psA", bufs=2, space="PSUM"))
        psW = ctx.enter_context(tc.tile_pool(name="psW", bufs=2, space="PSUM"))
        psB = ctx.enter_context(tc.tile_pool(name="psB", bufs=1, space="PSUM"))

        # ---- load inputs ----
        x_view = xp_d.rearrange("(kb p) f -> p kb f", p=128).rearrange(
            "p kb (h w) -> p kb h w", w=66)
        x_sb0 = sing.tile([128, 66, 66], BF16, tag="x_sb0")
        x_sb1 = sing.tile([128, 66, 66], BF16, tag="x_sb1")
        nc.sync.dma_start(out=x_sb0, in_=x_view[:, 0])
        nc.sync.dma_start(out=x_sb1, in_=x_view[:, 1])
        x_kb = (x_sb0, x_sb1)
        wt_sb = sing.tile([128, 2, 2, 9, 128], BF16, tag="wt_sb")
        nc.sync.dma_start(out=wt_sb, in_=wt_d.rearrange(
            "p (kb cv t c) -> p kb cv t c", kb=2, cv=2, t=9))
        cst = sing.tile([128, 10], F32, tag="cst")
        nc.sync.dma_start(out=cst, in_=cst_d[:, :])
        ind = sing.tile([128, 288], F32, tag="ind")
        nc.sync.dma_start(out=ind, in_=ind_d[:, :])
        cbf = sing.tile([128, 148], BF16, tag="cbf")
        nc.sync.dma_start(out=cbf, in_=cbf_d[:, :])

        ones_col = cbf[:, 128:129]          # [128,1] bf16 ones
        wpT = cbf[:, 129:148]               # [128,19] bf16 w_pred^T
        indg = ind[:, 0:32]                 # [128,32] group indicator
        indT = ind[0:32, 32:160]            # [32,128] transposed indicator

        # persistent feature buffers (ext = 64-col pad each side)
        exts32 = []
        extsbf = []
        extsT = []
        for name in ("pam", "cam"):
            exts32.append(sing.tile([128, NEXT], F32, tag=f"{name}_e32",
                                    name=f"{name}_e32"))
            extsbf.append(sing.tile([128, NEXT], BF16, tag=f"{name}_ebf",
                                    name=f"{name}_ebf"))
            extsT.append(sing.tile([128, NB, 128], BF16, tag=f"{name}_T",
                                   name=f"{name}_T"))

        # ---- conv + GN + ReLU for each head ----
        for cv in range(2):
            raw = work.tile([128, N], F32, tag="raw")
            stats = sm.tile([128, 8, 6], F32, tag="bnst")
            for nt in range(8):
                cps = psA.tile([128, 512], F32, tag="mm512")
                for kb in range(2):
                    for t in range(9):
                        dy, dx = t // 3, t % 3
                        rhs = x_kb[kb][:, nt * 8 + dy: nt * 8 + dy + 8, dx: dx + 64]
                        nc.tensor.matmul(
                            cps, wt_sb[:, kb, cv, t, :], rhs,
                            start=(kb == 0 and t == 0), stop=(kb == 1 and t == 8))
                nc.vector.tensor_copy(out=raw[:, nt * 512:(nt + 1) * 512], in_=cps)
                nc.vector.bn_stats(out=stats[:, nt, :], in_=cps)

            mv = sm.tile([128, 2], F32, tag="mv")
            nc.vector.bn_aggr(out=mv, in_=stats)
            t2 = sm.tile([128, 2], F32, tag="t2")
            nc.vector.tensor_copy(out=t2[:, 0:1], in_=mv[:, 0:1])
            m2 = sm.tile([128, 1], F32, tag="m2")
            nc.vector.tensor_mul(out=m2, in0=mv[:, 0:1], in1=mv[:, 0:1])
            nc.vector.tensor_add(out=t2[:, 1:2], in0=mv[:, 1:2], in1=m2)
            gps = psB.tile([32, 2], F32, tag="small")
            nc.tensor.matmul(gps, indg, t2, start=True, stop=True)
            gs = sm.tile([32, 2], F32, tag="gs")
            nc.vector.tensor_scalar_mul(gs, gps, 0.25)
            g2 = sm.tile([32, 2], F32, tag="g2")
            nc.vector.tensor_copy(out=g2[:, 0:1], in_=gs[:, 0:1])
            vg = sm.tile([32, 1], F32, tag="vg")
            nc.vector.tensor_mul(out=vg, in0=gs[:, 0:1], in1=gs[:, 0:1])
            nc.vector.tensor_sub(out=vg, in0=gs[:, 1:2], in1=vg)
            nc.scalar.activation(out=vg, in_=vg, func=AFT.Sqrt, bias=cst[0:32, 7:8])
            nc.vector.reciprocal(out=g2[:, 1:2], in_=vg)
            bcp = psB.tile([128, 2], F32, tag="small")
            nc.tensor.matmul(bcp, indT, g2, start=True, stop=True)
            bc = sm.tile([128, 2], F32, tag="bc")
            nc.vector.tensor_copy(out=bc, in_=bcp)
            ga = sm.tile([128, 1], F32, tag="ga")   # rstd*gamma
            nc.vector.tensor_mul(out=ga, in0=bc[:, 1:2],
                                 in1=cst[:, 2 + 2 * cv: 3 + 2 * cv])
            gb = sm.tile([128, 1], F32, tag="gb")   # beta - mean*rstd*gamma
            nc.vector.tensor_mul(out=gb, in0=bc[:, 0:1], in1=ga)
            nc.vector.tensor_sub(out=gb, in0=cst[:, 3 + 2 * cv: 4 + 2 * cv], in1=gb)

            e32 = exts32[cv]
            nc.scalar.activation(out=e32[:, PADN: PADN + N], in_=raw,
                                 func=AFT.Relu, bias=gb, scale=ga)
            # clamp pads (row 0 / row 63 duplicates)
            nc.vector.tensor_copy(out=e32[:, 0:PADN], in_=e32[:, PADN: 2 * PADN])
            nc.vector.tensor_copy(out=e32[:, PADN + N: NEXT],
                                  in_=e32[:, N: PADN + N])
            ebf = extsbf[cv]
            nc.vector.tensor_copy(out=ebf, in_=e32)
            # transposed copy of the center via PE transpose + DVE copyback
            eT = extsT[cv]
            for jb in range(NB):
                tp = psA.tile([128, 128], BF16, tag="mm512", name="tp")
                nc.tensor.transpose(tp, ebf[:, PADN + jb * 128: PADN + (jb + 1) * 128],
                                    cbf[:, 0:128])
                nc.vector.tensor_copy(out=eT[:, jb, :], in_=tp)

        pam32, cam32 = exts32
        pambf, cambf = extsbf
        pamT, camT = extsT

        # ---- CAM gram + row softmax ----
        gmp = psB.tile([128, 128], F32, tag="small")
        for jb in range(NB):
            nc.tensor.matmul(gmp, camT[:, jb, :], camT[:, jb, :],
                             start=(jb == 0), stop=(jb == NB - 1))
        gmx = sm.tile([128, 1], F32, tag="gmx")
        nc.vector.reduce_max(out=gmx, in_=gmp, axis=mybir.AxisListType.X)
        nc.vector.tensor_scalar_mul(gmx, gmx, -1.0)
        ge = sm.tile([128, 128], F32, tag="ge")
        gz = sm.tile([128, 1], F32, tag="gz")
        nc.scalar.activation(out=ge, in_=gmp, func=AFT.Exp, bias=gmx,
                             accum_out=gz)
        nc.vector.reciprocal(out=gz, in_=gz)
        camw = sm.tile([128, 128], BF16, tag="camw")
        nc.vector.tensor_scalar_mul(camw, ge, gz)
        camwT = sing.tile([128, 128], BF16, tag="camwT")
        tpw = psA.tile([128, 128], BF16, tag="mm512", name="tpw")
        nc.tensor.transpose(tpw, camw, cbf[:, 0:128])
        nc.vector.tensor_copy(out=camwT, in_=tpw)

        # ---- per-core window extraction (dynamic offset from partition id) ----
        pid = nc.partition_id(engines=(mybir.EngineType.DVE,))
        n0 = (pid % 4) * 1024
        qp_bf = sing.tile([128, NH], BF16, tag="qp_bf")
        qc_bf = sing.tile([128, NH], BF16, tag="qc_bf")
        qp32 = sing.tile([128, NH], F32, tag="qp32")
        qc32 = sing.tile([128, NH], F32, tag="qc32")
        nc.vector.tensor_copy(out=qp_bf, in_=pambf[:, ds(n0, NH)])
        nc.vector.tensor_copy(out=qc_bf, in_=cambf[:, ds(n0, NH)])
        nc.vector.tensor_copy(out=qp32, in_=pam32[:, ds(n0, NH)])
        nc.vector.tensor_copy(out=qc32, in_=cam32[:, ds(n0, NH)])

        # ---- PAM attention sweeps ----
        fused = sing.tile([128, NH], F32, tag="fused")
        base = sing.tile([128, NH], F32, tag="base")
        nc.vector.tensor_add(out=base, in0=qp32, in1=qc32)
        for off, wdt in SWEEPS:
            wps = psW.tile([128, 512], F32, tag="acc512")
            zps = psB.tile([1, 512], F32, tag="small")
            for jb in range(NB):
                sps = psA.tile([128, 512], F32, tag="mm512")
                nc.tensor.matmul(
                    sps[:, :wdt],
                    pambf[:, PADN + jb * 128: PADN + (jb + 1) * 128],
                    qp_bf[:, off: off + wdt], start=True, stop=True)
                pt = pt_pool.tile([128, 512], BF16, tag="pt")
                nc.scalar.activation(out=pt[:, :wdt], in_=sps[:, :wdt],
                                     func=AFT.Exp, bias=cst[:, 8:9])
                nc.tensor.matmul(wps[:, :wdt], pamT[:, jb, :], pt[:, :wdt],
                                 start=(jb == 0), stop=(jb == NB - 1))
                nc.tensor.matmul(zps[:, :wdt], ones_col, pt[:, :wdt],
                                 start=(jb == 0), stop=(jb == NB - 1))
            rz = sm.tile([1, 512], F32, tag="rz")
            nc.vector.reciprocal(out=rz[:, :wdt], in_=zps[:, :wdt])
            rzb = psA.tile([128, 512], F32, tag="mm512", name="rzb")
            nc.tensor.matmul(rzb[:, :wdt], ind[0:1, 160:288], rz[0:1, :wdt],
                             start=True, stop=True)
            rzs = pt_pool.tile([128, 512], F32, tag="rzs", name="rzs")
            nc.vector.tensor_copy(out=rzs[:, :wdt], in_=rzb[:, :wdt])
            nc.vector.tensor_mul(out=fused[:, off: off + wdt],
                                 in0=wps[:, :wdt], in1=rzs[:, :wdt])
        nc.vector.tensor_add(out=fused, in0=fused, in1=base)

        # ---- CAM weighted + fuse + predictor ----
        wcp = psB.tile([128, NH], F32, tag="big1152")
        for off, wdt in SWEEPS:
            nc.tensor.matmul(wcp[:, off: off + wdt], camwT,
                             qc_bf[:, off: off + wdt], start=True, stop=True)
        fbf = sing.tile([128, NH], BF16, tag="fbf")
        nc.vector.tensor_add(out=fbf, in0=fused, in1=wcp)
        lps = psB.tile([KC, NH], F32, tag="big1152")
        for off, wdt in SWEEPS:
            nc.tensor.matmul(lps[:, off: off + wdt], wpT,
                             fbf[:, off: off + wdt], start=True, stop=True)
        # logits + bias, padded cols (dup edge cols for clamp)
        lpad = sing.tile([KC, 18, 66], F32, tag="lpad")
        nc.vector.tensor_scalar_add(lpad[:, :, 1:65],
                                    lps.rearrange("k (r w) -> k r w", w=64),
                                    cst[0:KC, 6:7])
        nc.vector.tensor_copy(out=lpad[:, :, 0:1], in_=lpad[:, :, 1:2])
        nc.vector.tensor_copy(out=lpad[:, :, 65:66], in_=lpad[:, :, 64:65])

        # ---- bilinear x4 upsample ----
        # repA partitions: 4 groups of 32 (s*32 + k); rows shifted by s>=2
        repA = sing.tile([128, 17, 66], F32, tag="repA")
        nc.vector.memset(repA, 0.0)
        for s in range(4):
            o = 0 if s < 2 else 1
            nc.vector.tensor_copy(out=repA[32 * s: 32 * s + KC, :, :],
                                  in_=lpad[:, o: o + 17, :])
        r1a = sing.tile([128, 16, 66], F32, tag="r1a")
        r1b = sing.tile([128, 16, 66], F32, tag="r1b")
        nc.vector.tensor_scalar_mul(r1a, repA[:, 0:16, :], cst[:, 0:1])
        nc.vector.tensor_scalar_mul(r1b, repA[:, 1:17, :], cst[:, 1:2])
        nc.vector.tensor_add(out=r1a, in0=r1a, in1=r1b)
        for sp in range(4):
            o = 0 if sp < 2 else 1
            r2a = work.tile([128, 16, 64], F32, tag="r2a")
            r2b = work.tile([128, 16, 64], F32, tag="r2b")
            nc.vector.tensor_scalar_mul(r2a, r1a[:, :, o: o + 64], CA[sp])
            nc.vector.tensor_scalar_mul(r2b, r1a[:, :, o + 1: o + 65], CB[sp])
            nc.vector.tensor_add(out=r2a, in0=r2a, in1=r2b)
            for s in range(4):
                nc.sync.dma_start(
                    out=out_d[sp, s],
                    in_=r2a[32 * s: 32 * s + KC].rearrange("k q w -> k (q w)"))

    _split_multiwaits(nc)
    return nc


def _split_multiwaits(nc, max_waits=1):
    """This container's walrus codegen rejects instructions carrying more
    than one sync-wait (tail Drain aggregates one per live semaphore).
    Split extras into preceding single-wait Drains on the same engine."""
    import bass_rust
    n = 0
    for f in nc.m.functions:
        for b in f.blocks:
            new_list, changed = [], False
            for i in b.instructions:
                si = i.sync_info
                if si is not None and len(si.on_wait) > max_waits:
                    waits = list(si.on_wait)
                    for k, w in enumerate(waits[:-max_waits]):
                        d = mybir.InstNoOp(name=f"{i.name}-sw{k}")
                        d.engine = i.engine
                        d.sync_info = bass_rust.SyncInfo(on_update=[], on_wait=[w])
                        try:
                            nc.register_instruction(d, overwrite=True)
                        except Exception:
                            pass
                        new_list.append(d)
                        n += 1
                    si.on_wait = waits[-max_waits:]
                    changed = True
                new_list.append(i)
            if changed:
                b.instructions = new_list
    return n


_CACHE = {}


def _get_nc():
    if "nc" not in _CACHE:
        _CACHE["nc"] = _build()
    return _CACHE["nc"]


def _host_inputs(x, w_pam, gn_pam_scale, gn_pam_bias, w_cam, gn_cam_scale,
                 gn_cam_bias, w_pred, b_pred):
    bf = ml_dtypes.bfloat16
    # x padded with conv zero-pad, per sample
    xps = []
    for b in range(B):
        xp = np.zeros((CIN, 66, 66), np.float32)
        xp[:, 1:65, 1:65] = x[b]
        xps.append(xp.reshape(CIN, 66 * 66).astype(bf))
    # weights: wt[p, kb, cv, t, cd] = w_cv[cd, kb*128+p, dy, dx]
    wt = np.zeros((128, 2, 2, 9, 128), np.float32)
    for cv, wconv in enumerate((w_pam, w_cam)):
        wr = wconv.reshape(CD, 2, 128, 3, 3)  # cd, kb, p, dy, dx
        wt[:, :, cv, :, :] = wr.transpose(2, 1, 3, 4, 0).reshape(128, 2, 9, 128)
    wt = wt.reshape(128, -1).astype(bf)

    cst = np.zeros((128, 10), np.float32)
    cst[:, 7] = EPS
    cst[:, 8] = -SHIFT
    for s in range(4):
        cst[32 * s: 32 * s + 32, 0] = C0[s]
        cst[32 * s: 32 * s + 32, 1] = C1[s]
    cst[:, 2] = gn_pam_scale
    cst[:, 3] = gn_pam_bias
    cst[:, 4] = gn_cam_scale
    cst[:, 5] = gn_cam_bias
    cst[:KC, 6] = b_pred

    ind = np.zeros((128, 288), np.float32)
    for p in range(128):
        ind[p, p // 4] = 1.0
    for g in range(32):
        ind[g, 32 + 4 * g: 32 + 4 * g + 4] = 1.0
    ind[0, 160:288] = 1.0

    cbfm = np.zeros((128, 148), np.float32)
    cbfm[:, :128] = np.eye(128)
    cbfm[:, 128] = 1.0
    cbfm[:, 129:148] = w_pred[:, :, 0, 0].T
    cbfm = cbfm.astype(bf)

    maps = []
    for core in range(8):
        maps.append({"xp": xps[core // 4], "wt": wt, "cst": cst,
                     "ind": ind, "cbf": cbfm})
    return maps


def _assemble(results):
    out = np.zeros((B, KC, 256, 256), np.float32)
    for core in range(8):
        b, r = core // 4, core % 4
        buf = results[core]["outb"]  # [4 s', 4 s, KC, 1024]
        arr = buf.reshape(4, 4, KC, 16, 64)
        # out[b, k, 64r + 4q + s, 4p + s'] = arr[s', s, k, q, p]
        out[b, :, 64 * r: 64 * (r + 1), :] = (
            arr.transpose(2, 3, 1, 4, 0).reshape(KC, 64, 256))
    return out


def kernel(**inputs):
    from concourse.bass_utils import run_bass_kernel_spmd
    nc = _get_nc()
    in_maps = _host_inputs(**{k: np.asarray(v) for k, v in inputs.items()})
    trace = bool(int(os.environ.get("KERNEL_TRACE", "0")))
    res = run_bass_kernel_spmd(nc, in_maps, core_ids=list(range(8)), trace=trace)
    _CACHE["last_result"] = res
    return _assemble(res.results)


if __name__ == "__main__":
    # CoreSim smoke test of a single core (core given by argv[1], default 0)
    core = int(sys.argv[1]) if len(sys.argv) > 1 else 0
    sys.path.insert(0, "/root/problem")
    os.environ.setdefault("JAX_PLATFORMS", "cpu")
    import reference as R
    inputs = {k: np.asarray(v) for k, v in R.setup_inputs().items()}
    nc = _get_nc()
    maps = _host_inputs(**inputs)
    from concourse.bass_interp import CoreSim
    sim = CoreSim(nc)
    for k, v in maps[core].items():
        sim.tensor(k)[:] = v
    sim.tensor("partition_id")[:] = np.array([[core]], np.uint32)
    sim.simulate()
    print(f"sim time: {sim.time} ns")
    buf = np.array(sim.tensor("outb"))
    ref = np.asarray(R.reference(**inputs))
    b, r = core // 4, core % 4
    arr = buf.reshape(4, 4, KC, 16, 64)
    mine = arr.transpose(2, 3, 1, 4, 0).reshape(KC, 64, 256)
    refs = ref[b, :, 64 * r: 64 * (r + 1), :]
    err = np.abs(mine - refs).max()
    scale = np.abs(refs).max()
    print(f"core {core}: absmax err {err:.5f}  scale {scale:.3f}  "
          f"rel {err / scale:.5f}")
